# revision 1
# baseline (speedup 1.0000x reference)
"""GQA attention kernel for Trainium2, 8-core tensor-parallel (by heads).

Shapes (hardcoded from the problem spec):
  x:(4,128,4096) fp32, wq:(4096,4096), wk/wv:(4096,1024), wo:(4096,4096),
  32 q heads / 8 kv heads, head_dim 128, start_pos=0 (cache is overwritten).

Sharding: core c owns q heads [4c,4c+4) and kv head c; wq/wk/wv column-
sharded, wo row-sharded; each core computes a full (512,4096) partial of
the output projection; host sums the 8 partials and adds bo.
"""
import sys
sys.path.insert(0, "/opt/trn_rl_repo")

import numpy as np

B, S, D = 4, 128, 4096
H, KV, HD = 32, 8, 128
NCORES = 8
HQ = H // NCORES          # 4 q heads per core
T = B * S                 # 512 tokens
FQ = HQ * HD              # 512 q features per core
SCALE = 1.0 / float(np.sqrt(HD))

_CACHE = {}


def _build():
    import concourse.bass as bass
    import concourse.tile as tile
    from concourse import bacc, mybir

    F32, F32R = mybir.dt.float32, mybir.dt.float32r
    AF = mybir.ActivationFunctionType

    nc = bacc.Bacc("TRN2", target_bir_lowering=False, debug=False,
                   enable_asserts=False, num_devices=NCORES)

    xT_d = nc.dram_tensor("xT", [D, T], F32R, kind="ExternalInput").ap()
    wq_d = nc.dram_tensor("wq", [D, FQ], F32R, kind="ExternalInput").ap()
    wkv_d = nc.dram_tensor("wkv", [D, 2 * HD], F32R, kind="ExternalInput").ap()
    wo_d = nc.dram_tensor("wo", [FQ, D], F32R, kind="ExternalInput").ap()
    bq_d = nc.dram_tensor("bq", [1, FQ], F32, kind="ExternalInput").ap()
    bkv_d = nc.dram_tensor("bkv", [1, 2 * HD], F32, kind="ExternalInput").ap()
    c4_d = nc.dram_tensor("c4", [S, HQ * 64], F32, kind="ExternalInput").ap()
    s4_d = nc.dram_tensor("s4", [S, HQ * 64], F32, kind="ExternalInput").ap()
    mk_d = nc.dram_tensor("mk", [S, HQ * S], F32, kind="ExternalInput").ap()
    on_d = nc.dram_tensor("on", [S, S], F32, kind="ExternalInput").ap()
    id_d = nc.dram_tensor("idm", [S, S], F32, kind="ExternalInput").ap()
    out_d = nc.dram_tensor("out", [T, D], F32, kind="ExternalOutput").ap()

    NK = D // 128   # 32 contraction chunks

    with tile.TileContext(nc) as tc:
        with tc.tile_pool(name="consts", bufs=1) as cp:
            b128q = cp.tile([128, FQ], F32)
            b128kv = cp.tile([128, 2 * HD], F32)
            c4 = cp.tile([128, HQ * 64], F32)
            s4 = cp.tile([128, HQ * 64], F32)
            mk = cp.tile([128, HQ * S], F32)
            ones = cp.tile([128, S], F32)
            ident = cp.tile([128, S], F32)
            nc.gpsimd.dma_start(b128q, bass.AP(tensor=bq_d.tensor, offset=0,
                                               ap=[[0, 128], bq_d.ap[1]]))
            nc.gpsimd.dma_start(b128kv, bass.AP(tensor=bkv_d.tensor, offset=0,
                                                ap=[[0, 128], bkv_d.ap[1]]))
            nc.sync.dma_start(c4, c4_d)
            nc.sync.dma_start(s4, s4_d)
            nc.sync.dma_start(mk, mk_d)
            nc.sync.dma_start(ones, on_d)
            nc.sync.dma_start(ident, id_d)

            with tc.tile_pool(name="qkvs", bufs=4) as qp, \
                 tc.tile_pool(name="ropep", bufs=4) as rp, \
                 tc.tile_pool(name="tmpp", bufs=2) as tp, \
                 tc.tile_pool(name="trs", bufs=4) as trp, \
                 tc.tile_pool(name="attn", bufs=2) as ap_, \
                 tc.tile_pool(name="aop", bufs=4) as aop, \
                 tc.tile_pool(name="outp", bufs=8) as op:

                # ---------------- Phase A: QKV projections -------------
                q_sb = [None] * B
                kv_sb = [None] * B
                with tc.tile_pool(name="psA", bufs=4, space="PSUM") as psA, \
                     tc.tile_pool(name="xtp", bufs=10) as xp, \
                     tc.tile_pool(name="wp", bufs=10) as wp:
                    pq = [psA.tile([128, FQ], F32, tag="pq", name=f"pq{m}")
                          for m in range(B)]
                    pkv = [psA.tile([128, 2 * HD], F32, tag="pkv", name=f"pkv{m}")
                           for m in range(B)]
                    for k in range(NK):
                        xt = xp.tile([128, T], F32R, tag="xt", name=f"xt{k}")
                        (nc.sync if k % 2 == 0 else nc.scalar).dma_start(xt, xT_d[k * 128:(k + 1) * 128, :])
                        wqt = wp.tile([128, FQ], F32R, tag="wqt", name=f"wqt{k}")
                        (nc.scalar if k % 2 == 0 else nc.sync).dma_start(wqt, wq_d[k * 128:(k + 1) * 128, :])
                        wkvt = wp.tile([128, 2 * HD], F32R, tag="wkvt", name=f"wkvt{k}")
                        nc.scalar.dma_start(wkvt, wkv_d[k * 128:(k + 1) * 128, :])
                        for m in range(B):
                            lhs = xt[:, m * 128:(m + 1) * 128]
                            nc.tensor.matmul(pq[m], lhs, wqt,
                                             start=(k == 0), stop=(k == NK - 1))
                            nc.tensor.matmul(pkv[m], lhs, wkvt,
                                             start=(k == 0), stop=(k == NK - 1))
                    for m in range(B):
                        q_sb[m] = qp.tile([128, FQ], F32, tag="q", name=f"q{m}")
                        nc.vector.tensor_add(q_sb[m], pq[m], b128q)
                        kv_sb[m] = qp.tile([128, 2 * HD], F32, tag="kv", name=f"kv{m}")
                        nc.vector.tensor_add(kv_sb[m], pkv[m], b128kv)

                # ------------- Phases B-D per batch tile ---------------
                with tc.tile_pool(name="psB", bufs=1, space="PSUM") as psB, \
                     tc.tile_pool(name="wop", bufs=16) as wop:
                    c4v = c4.rearrange("p (h r) -> p h r", h=HQ)
                    s4v = s4.rearrange("p (h r) -> p h r", h=HQ)
                    aoT = [None] * B
                    for m in range(B):
                        # RoPE on q
                        qv = q_sb[m].rearrange("p (h r two) -> p h r two",
                                               h=HQ, r=64, two=2)
                        q_e, q_o = qv[:, :, :, 0], qv[:, :, :, 1]
                        qr = rp.tile([128, FQ], F32, tag="qr", name=f"qr{m}")
                        qrv = qr.rearrange("p (h r two) -> p h r two",
                                           h=HQ, r=64, two=2)
                        t1 = tp.tile([128, HQ * 64], F32, tag="t1", name=f"t1_{m}")
                        t2 = tp.tile([128, HQ * 64], F32, tag="t2", name=f"t2_{m}")
                        t1v = t1.rearrange("p (h r) -> p h r", h=HQ)
                        t2v = t2.rearrange("p (h r) -> p h r", h=HQ)
                        nc.vector.tensor_mul(t1v, q_o, s4v)
                        nc.vector.tensor_mul(t2v, q_e, c4v)
                        nc.vector.tensor_sub(qrv[:, :, :, 0], t2v, t1v)
                        nc.vector.tensor_mul(t1v, q_o, c4v)
                        nc.vector.tensor_mul(t2v, q_e, s4v)
                        nc.vector.tensor_add(qrv[:, :, :, 1], t2v, t1v)
                        # RoPE on k (head 0 of kv tile)
                        kv_ = kv_sb[m][:, 0:HD].rearrange("p (r two) -> p r two",
                                                          r=64, two=2)
                        k_e, k_o = kv_[:, :, 0], kv_[:, :, 1]
                        kr = rp.tile([128, HD], F32, tag="kr", name=f"kr{m}")
                        krv = kr.rearrange("p (r two) -> p r two", r=64, two=2)
                        t1k = t1v[:, 0, :]
                        t2k = t2v[:, 0, :]
                        c1 = c4v[:, 0, :]
                        s1 = s4v[:, 0, :]
                        nc.vector.tensor_mul(t1k, k_o, s1)
                        nc.vector.tensor_mul(t2k, k_e, c1)
                        nc.vector.tensor_sub(krv[:, :, 0], t2k, t1k)
                        nc.vector.tensor_mul(t1k, k_o, c1)
                        nc.vector.tensor_mul(t2k, k_e, s1)
                        nc.vector.tensor_add(krv[:, :, 1], t2k, t1k)

                        # Transposes -> qT [d,(h,i)], kT [d,j]
                        pstq = psB.tile([128, FQ], F32, tag="pstq", name=f"pstq{m}")
                        for h in range(HQ):
                            nc.tensor.transpose(pstq[:, h * 128:(h + 1) * 128],
                                                qr[:, h * 128:(h + 1) * 128], ident)
                        qT = trp.tile([128, FQ], F32R, tag="qT", name=f"qT{m}")
                        nc.vector.tensor_copy(qT, pstq)
                        pstk = psB.tile([128, HD], F32, tag="pstk", name=f"pstk{m}")
                        nc.tensor.transpose(pstk, kr, ident)
                        kT = trp.tile([128, HD], F32R, tag="kT", name=f"kT{m}")
                        nc.scalar.copy(kT, pstk)

                        # Attention (scoresT layout [j,(h,i)])
                        psc = psB.tile([128, FQ], F32, tag="psc", bufs=2, name=f"psc{m}")
                        nc.tensor.matmul(psc, kT, qT, start=True, stop=True)
                        expT = ap_.tile([128, FQ], F32, tag="expT", name=f"expT{m}")
                        nc.scalar.activation(expT, psc, AF.Exp, scale=SCALE)
                        attn_u = ap_.tile([128, FQ], F32, tag="attn_u", name=f"au{m}")
                        nc.vector.tensor_mul(attn_u, expT, mk)
                        pden = psB.tile([128, FQ], F32, tag="pden", name=f"pden{m}")
                        nc.tensor.matmul(pden, ones, attn_u, start=True, stop=True)
                        rec = ap_.tile([128, FQ], F32, tag="rec", name=f"rec{m}")
                        nc.vector.reciprocal(rec, pden)
                        attn_n = ap_.tile([128, FQ], F32, tag="attn_n", name=f"an{m}")
                        nc.vector.tensor_mul(attn_n, attn_u, rec)
                        poT = psB.tile([128, FQ], F32, tag="poT", name=f"poT{m}")
                        nc.tensor.matmul(poT, kv_sb[m][:, HD:2 * HD], attn_n,
                                         start=True, stop=True)
                        aoT[m] = aop.tile([128, FQ], F32R, tag="aoT", name=f"aoT{m}")
                        nc.vector.tensor_copy(aoT[m], poT)

                    # ---------------- Phase D: output projection ------------
                    NT = D // 512  # 8 column tiles
                    for n in range(NT):
                        wts = []
                        for h in range(HQ):
                            wt = wop.tile([128, 512], F32R, tag="wo", name=f"wo{n}_{h}")
                            nc.scalar.dma_start(
                                wt, wo_d[h * 128:(h + 1) * 128,
                                         n * 512:(n + 1) * 512])
                            wts.append(wt)
                        for m in range(B):
                            pso = psB.tile([128, 512], F32, tag="pso", bufs=2,
                                           name=f"pso{n}_{m}")
                            for h in range(HQ):
                                nc.tensor.matmul(pso, aoT[m][:, h * 128:(h + 1) * 128],
                                                 wts[h], start=(h == 0),
                                                 stop=(h == HQ - 1))
                            osb = op.tile([128, 512], F32, tag="osb",
                                          name=f"osb{n}_{m}")
                            if (n * B + m) % 2 == 0:
                                nc.vector.tensor_copy(osb, pso)
                            else:
                                nc.scalar.copy(osb, pso)
                            nc.sync.dma_start(
                                out_d[m * 128:(m + 1) * 128,
                                      n * 512:(n + 1) * 512], osb)

    nc.compile()
    return nc


def _prep_inputs(x, freqs_cos, freqs_sin, wq, bq, wk, bk, wv, bv, wo):
    xT = np.ascontiguousarray(x.reshape(T, D).T.astype(np.float32))
    c4 = np.ascontiguousarray(np.tile(freqs_cos.astype(np.float32), (1, HQ)))
    s4 = np.ascontiguousarray(np.tile(freqs_sin.astype(np.float32), (1, HQ)))
    mk = np.ascontiguousarray(
        np.tile(np.triu(np.ones((S, S), np.float32)), (1, HQ)))
    on = np.ones((S, S), np.float32)
    idm = np.eye(S, dtype=np.float32)
    maps = []
    for c in range(NCORES):
        qs = slice(c * FQ, (c + 1) * FQ)
        ks = slice(c * HD, (c + 1) * HD)
        maps.append({
            "xT": xT,
            "wq": np.ascontiguousarray(wq[:, qs].astype(np.float32)),
            "wkv": np.ascontiguousarray(
                np.concatenate([wk[:, ks], wv[:, ks]], axis=1).astype(np.float32)),
            "wo": np.ascontiguousarray(wo[qs, :].astype(np.float32)),
            "bq": np.ascontiguousarray(bq[qs].astype(np.float32)).reshape(1, FQ),
            "bkv": np.ascontiguousarray(
                np.concatenate([bk[ks], bv[ks]]).astype(np.float32)).reshape(1, 2 * HD),
            "c4": c4, "s4": s4, "mk": mk, "on": on, "idm": idm,
        })
    return maps


def kernel(x, start_pos, freqs_cos, freqs_sin, mask, cache_k, cache_v,
           wq, bq, wk, bk, wv, bv, wo, bo, _want_trace=False):
    from concourse.bass_utils import run_bass_kernel_spmd

    assert int(start_pos) == 0
    if "nc" not in _CACHE:
        _CACHE["nc"] = _build()
    nc = _CACHE["nc"]
    in_maps = _prep_inputs(np.asarray(x), np.asarray(freqs_cos),
                           np.asarray(freqs_sin), np.asarray(wq),
                           np.asarray(bq), np.asarray(wk), np.asarray(bk),
                           np.asarray(wv), np.asarray(bv), np.asarray(wo))
    res = run_bass_kernel_spmd(nc, in_maps, core_ids=list(range(NCORES)),
                               trace=_want_trace)
    acc = np.zeros((T, D), np.float64)
    for r in res.results:
        acc += r["out"].astype(np.float64)
    out = (acc + np.asarray(bo).astype(np.float64)).astype(np.float32)
    if _want_trace:
        _CACHE["last_exec_time_ns"] = res.exec_time_ns
        _CACHE["last_trace"] = res.instructions_and_trace
    return out.reshape(B, S, D)



# revision 2
# speedup vs baseline: 39.1601x; 39.1601x over previous
"""GQA attention kernel v2 for Trainium2, 8-core tensor-parallel (by heads).

Same sharding as v1 (core c owns q heads [4c,4c+4) and kv head c), but:
  - all big tensors (x, wq/wk/wv, wo, out) in bf16 -> half the HBM traffic
  - QKV projections computed directly in transposed [feat, token] layout
    (weights stationary, x moving) so q/k/v need no per-head PE transposes
  - RoPE applied in transposed layout via a pair-swap permutation matmul
    plus elementwise cos/sin multiplies
  - output partials stored bf16; host sums the 8 partials and adds bo
"""
import sys
sys.path.insert(0, "/opt/trn_rl_repo")

import numpy as np

B, S, D = 4, 128, 4096
H, KV, HD = 32, 8, 128
NCORES = 8
HQ = H // NCORES          # 4 q heads per core
T = B * S                 # 512 tokens
FQ = HQ * HD              # 512 q features per core
NK = D // 128             # 32 contraction chunks
SCALE = 1.0 / float(np.sqrt(HD))

_CACHE = {}


def _build():
    import concourse.bass as bass
    import concourse.tile as tile
    from concourse import bacc, mybir

    F32 = mybir.dt.float32
    BF16 = mybir.dt.bfloat16
    AF = mybir.ActivationFunctionType

    nc = bacc.Bacc("TRN2", target_bir_lowering=False, debug=False,
                   enable_asserts=False, num_devices=NCORES)

    xs_d = nc.dram_tensor("xs", [128, NK * T], BF16, kind="ExternalInput").ap()
    wqkv_d = nc.dram_tensor("wqkv", [128, NK * 768], BF16, kind="ExternalInput").ap()
    wo_d = nc.dram_tensor("wo", [FQ, D], BF16, kind="ExternalInput").ap()
    cosT_d = nc.dram_tensor("cosT", [128, T], BF16, kind="ExternalInput").ap()
    sinT_d = nc.dram_tensor("sinT", [128, T], BF16, kind="ExternalInput").ap()
    mkT_d = nc.dram_tensor("mkT", [128, HQ * S], BF16, kind="ExternalInput").ap()
    ones_d = nc.dram_tensor("ones", [128, S], BF16, kind="ExternalInput").ap()
    identT_d = nc.dram_tensor("identT", [128, S], BF16, kind="ExternalInput").ap()
    pswap_d = nc.dram_tensor("pswap", [128, S], BF16, kind="ExternalInput").ap()
    bqT_d = nc.dram_tensor("bqT", [128, HQ], F32, kind="ExternalInput").ap()
    bkvT_d = nc.dram_tensor("bkvT", [128, 2], F32, kind="ExternalInput").ap()
    out_d = nc.dram_tensor("out", [T, D], BF16, kind="ExternalOutput").ap()

    # k-chunk DMA group sizes: small first groups so PE starts early
    GROUPS = [1, 1, 2, 4, 6, 6, 6, 6]
    assert sum(GROUPS) == NK

    with tile.TileContext(nc) as tc:
        with tc.tile_pool(name="consts", bufs=1) as cp:
            xs = cp.tile([128, NK * T], BF16)
            wq_s = cp.tile([128, NK * 768], BF16)
            wo_s = [cp.tile([128, D], BF16, name=f"wo{h}") for h in range(HQ)]
            cosT = cp.tile([128, T], BF16)
            sinT = cp.tile([128, T], BF16)
            mkT = cp.tile([128, HQ * S], BF16)
            ones = cp.tile([128, S], BF16)
            identT = cp.tile([128, S], BF16)
            pswap = cp.tile([128, S], BF16)
            bqT = cp.tile([128, HQ], F32)
            bkvT = cp.tile([128, 2], F32)

            # weights stream on the gpsimd (SWDGE) queue, x on the sync
            # (HWDGE-SP) queue; consts on scalar (HWDGE-ACT). Output stores
            # reuse SP later.
            k0 = 0
            for g, kg in enumerate(GROUPS):
                nc.gpsimd.dma_start(wq_s[:, k0 * 768:(k0 + kg) * 768],
                                    wqkv_d[:, k0 * 768:(k0 + kg) * 768])
                nc.sync.dma_start(xs[:, k0 * T:(k0 + kg) * T],
                                  xs_d[:, k0 * T:(k0 + kg) * T])
                k0 += kg
            for h in range(HQ):
                nc.gpsimd.dma_start(wo_s[h], wo_d[h * 128:(h + 1) * 128, :])
            nc.scalar.dma_start(cosT, cosT_d)
            nc.scalar.dma_start(sinT, sinT_d)
            nc.scalar.dma_start(mkT, mkT_d)
            nc.scalar.dma_start(ones, ones_d)
            nc.scalar.dma_start(identT, identT_d)
            nc.scalar.dma_start(pswap, pswap_d)
            nc.scalar.dma_start(bqT, bqT_d)
            nc.scalar.dma_start(bkvT, bkvT_d)

            with tc.tile_pool(name="sb", bufs=1) as sp, \
                 tc.tile_pool(name="tmp", bufs=2) as tp, \
                 tc.tile_pool(name="attn", bufs=2) as ap_, \
                 tc.tile_pool(name="aop", bufs=4) as aop, \
                 tc.tile_pool(name="outp", bufs=4) as op:

                # ---------- Phase A: QKV projections (transposed out) ------
                qTs = [sp.tile([128, T], BF16, name=f"qT{h}") for h in range(HQ)]
                kT_pre = sp.tile([128, T], BF16, name="kT_pre")
                vT = sp.tile([128, T], BF16, name="vT")
                with tc.tile_pool(name="psA", bufs=1, space="PSUM") as psA:
                    pq = [psA.tile([128, T], F32, tag=f"pq{h}", name=f"pq{h}")
                          for h in range(HQ)]
                    pk = psA.tile([128, T], F32, tag="pk", name="pk")
                    pv = psA.tile([128, T], F32, tag="pv", name="pv")
                    for k in range(NK):
                        rhs = xs[:, k * T:(k + 1) * T]
                        base = k * 768
                        st = (k == 0)
                        sp_ = (k == NK - 1)
                        for h in range(HQ):
                            nc.tensor.matmul(
                                pq[h], wq_s[:, base + h * 128:base + (h + 1) * 128],
                                rhs, start=st, stop=sp_)
                        nc.tensor.matmul(pk, wq_s[:, base + 512:base + 640],
                                         rhs, start=st, stop=sp_)
                        nc.tensor.matmul(pv, wq_s[:, base + 640:base + 768],
                                         rhs, start=st, stop=sp_)
                    # PSUM -> SBUF with bias add + bf16 cast (ACT/DVE split)
                    for h in range(HQ):
                        if h % 2 == 0:
                            nc.scalar.activation(qTs[h], pq[h],
                                                 AF.Identity, bias=bqT[:, h:h + 1])
                        else:
                            nc.vector.tensor_scalar_add(qTs[h], pq[h],
                                                        bqT[:, h:h + 1])
                    nc.vector.tensor_scalar_add(vT, pv, bkvT[:, 1:2])
                    nc.scalar.activation(kT_pre, pk, AF.Identity, bias=bkvT[:, 0:1])

                # ---------- Phase B: RoPE (transposed) + v transposes ------
                qrT = sp.tile([128, HQ * T], BF16, name="qrT")
                krT = sp.tile([128, T], BF16, name="krT")
                v_m = [None] * B
                with tc.tile_pool(name="psB", bufs=1, space="PSUM") as psB:
                    for h in range(HQ + 1):
                        src = (qTs[h] if h < HQ else kT_pre)
                        dst = (qrT[:, h * T:(h + 1) * T] if h < HQ else krT)
                        pswp = psB.tile([128, T], F32, tag="pswp", bufs=2,
                                        name=f"pswp{h}")
                        nc.tensor.matmul(pswp, pswap, src, start=True, stop=True)
                        t1 = tp.tile([128, T], BF16, tag="t1", name=f"t1_{h}")
                        nc.vector.tensor_mul(t1, src, cosT)
                        t2 = tp.tile([128, T], BF16, tag="t2", name=f"t2_{h}")
                        nc.vector.tensor_mul(t2, pswp, sinT)
                        nc.vector.tensor_add(dst, t1, t2)
                    for m in range(B):
                        pvm = psB.tile([128, S], BF16, tag="pvm", bufs=1,
                                       name=f"pvm{m}")
                        nc.tensor.transpose(pvm, vT[:, m * S:(m + 1) * S], identT)
                        v_m[m] = sp.tile([128, S], BF16, name=f"v{m}")
                        nc.vector.tensor_copy(v_m[m], pvm)

                    # ---------- Phase C: attention per batch ----------
                    # po_u = V^T @ (exp(scores) * mask) runs in parallel with
                    # the denominator matmul; normalization folds into the
                    # PSUM->SBUF copy (rec rows are all equal).
                    qv = qrT.rearrange("p (h t) -> p h t", h=HQ)
                    aoT = [None] * B
                    for m in range(B):
                        psc = psB.tile([128, HQ * S], F32, tag="psc",
                                       name=f"psc{m}")
                        nc.tensor.matmul(psc, krT[:, m * S:(m + 1) * S],
                                         qv[:, :, m * S:(m + 1) * S],
                                         start=True, stop=True)
                        eu = ap_.tile([128, HQ * S], BF16, tag="eu", name=f"eu{m}")
                        nc.scalar.activation(eu, psc, AF.Exp, scale=SCALE)
                        au = ap_.tile([128, HQ * S], BF16, tag="au", name=f"au{m}")
                        nc.gpsimd.tensor_mul(au, eu, mkT)
                        pden = psB.tile([128, HQ * S], F32, tag="pden",
                                        name=f"pden{m}")
                        nc.tensor.matmul(pden, ones, au, start=True, stop=True)
                        po = psB.tile([128, HQ * S], F32, tag="po", name=f"po{m}")
                        nc.tensor.matmul(po, v_m[m], au, start=True, stop=True)
                        rec = ap_.tile([128, HQ * S], F32, tag="rec", name=f"rec{m}")
                        nc.vector.reciprocal(rec, pden)
                        aoT[m] = aop.tile([128, HQ * S], BF16, tag="aoT",
                                          name=f"aoT{m}")
                        nc.vector.tensor_mul(aoT[m], po, rec)

                    # ---------- Phase D: output projection ----------
                    NT = D // 512
                    for m in range(B):
                        outm = op.tile([128, D], BF16, tag="outm", name=f"outm{m}")
                        for n in range(NT):
                            pso = psB.tile([128, 512], F32, tag="pso", bufs=2,
                                           name=f"pso{m}_{n}")
                            for h in range(HQ):
                                nc.tensor.matmul(
                                    pso, aoT[m][:, h * 128:(h + 1) * 128],
                                    wo_s[h][:, n * 512:(n + 1) * 512],
                                    start=(h == 0), stop=(h == HQ - 1))
                            if (m * NT + n) % 2 == 0:
                                nc.vector.tensor_copy(
                                    outm[:, n * 512:(n + 1) * 512], pso)
                            else:
                                nc.scalar.copy(
                                    outm[:, n * 512:(n + 1) * 512], pso)
                            if n % 2 == 1:
                                # store finished 1024-col quarter immediately
                                qlo = (n - 1) * 512
                                nc.sync.dma_start(
                                    out_d[m * S:(m + 1) * S, qlo:qlo + 1024],
                                    outm[:, qlo:qlo + 1024])

    nc.compile()
    return nc


def _prep_inputs(x, freqs_cos, freqs_sin, wq, bq, wk, bk, wv, bv, wo):
    from ml_dtypes import bfloat16 as bf16

    xT = x.reshape(T, D).T.astype(np.float32)                  # (D, T)
    xs = np.ascontiguousarray(
        xT.reshape(NK, 128, T).transpose(1, 0, 2).reshape(128, NK * T)
    ).astype(bf16)

    cos_d = np.repeat(freqs_cos.astype(np.float32), 2, axis=1)  # (S, 128)
    sin_d = np.repeat(freqs_sin.astype(np.float32), 2, axis=1)
    sign = np.tile(np.array([-1.0, 1.0], np.float32), HD // 2)
    cosT = np.ascontiguousarray(np.tile(cos_d.T, (1, B))).astype(bf16)  # (128, T)
    sinT = np.ascontiguousarray(
        np.tile((sin_d * sign[None, :]).T, (1, B))).astype(bf16)
    mkT = np.ascontiguousarray(
        np.tile(np.triu(np.ones((S, S), np.float32)), (1, HQ))).astype(bf16)
    ones = np.ones((S, S), np.float32).astype(bf16)
    identT = np.eye(S, dtype=np.float32).astype(bf16)
    pswap = np.kron(np.eye(HD // 2, dtype=np.float32),
                    np.array([[0, 1], [1, 0]], np.float32)).astype(bf16)

    maps = []
    for c in range(NCORES):
        qs = slice(c * FQ, (c + 1) * FQ)
        ks = slice(c * HD, (c + 1) * HD)
        wqkv = np.concatenate(
            [wq[:, qs], wk[:, ks], wv[:, ks]], axis=1).astype(np.float32)  # (D, 768)
        wqkv_t = np.ascontiguousarray(
            wqkv.reshape(NK, 128, 768).transpose(1, 0, 2).reshape(128, NK * 768)
        ).astype(bf16)
        bqT = np.ascontiguousarray(
            bq[qs].astype(np.float32).reshape(HQ, HD).T)       # (128, HQ)
        bkvT = np.ascontiguousarray(
            np.stack([bk[ks], bv[ks]], axis=1).astype(np.float32))  # (128, 2)
        maps.append({
            "xs": xs,
            "wqkv": wqkv_t,
            "wo": np.ascontiguousarray(wo[qs, :].astype(np.float32)).astype(bf16),
            "cosT": cosT, "sinT": sinT, "mkT": mkT, "ones": ones,
            "identT": identT, "pswap": pswap, "bqT": bqT, "bkvT": bkvT,
        })
    return maps


def kernel(x, start_pos, freqs_cos, freqs_sin, mask, cache_k, cache_v,
           wq, bq, wk, bk, wv, bv, wo, bo, _want_trace=False):
    from concourse.bass_utils import run_bass_kernel_spmd

    assert int(start_pos) == 0
    if "nc" not in _CACHE:
        _CACHE["nc"] = _build()
    nc = _CACHE["nc"]
    in_maps = _prep_inputs(np.asarray(x), np.asarray(freqs_cos),
                           np.asarray(freqs_sin), np.asarray(wq),
                           np.asarray(bq), np.asarray(wk), np.asarray(bk),
                           np.asarray(wv), np.asarray(bv), np.asarray(wo))
    res = run_bass_kernel_spmd(nc, in_maps, core_ids=list(range(NCORES)),
                               trace=_want_trace)
    acc = np.zeros((T, D), np.float64)
    for r in res.results:
        acc += r["out"].astype(np.float64)
    out = (acc + np.asarray(bo).astype(np.float64)).astype(np.float32)
    if _want_trace:
        _CACHE["last_exec_time_ns"] = res.exec_time_ns
        _CACHE["last_trace"] = res.instructions_and_trace
    return out.reshape(B, S, D)


# revision 3
# speedup vs baseline: 40.7756x; 1.0413x over previous
"""GQA attention kernel v2 for Trainium2, 8-core tensor-parallel (by heads).

Same sharding as v1 (core c owns q heads [4c,4c+4) and kv head c), but:
  - all big tensors (x, wq/wk/wv, wo, out) in bf16 -> half the HBM traffic
  - QKV projections computed directly in transposed [feat, token] layout
    (weights stationary, x moving) so q/k/v need no per-head PE transposes
  - RoPE applied in transposed layout via a pair-swap permutation matmul
    plus elementwise cos/sin multiplies
  - output partials stored bf16; host sums the 8 partials and adds bo
"""
import sys
sys.path.insert(0, "/opt/trn_rl_repo")

import numpy as np

B, S, D = 4, 128, 4096
H, KV, HD = 32, 8, 128
NCORES = 8
HQ = H // NCORES          # 4 q heads per core
T = B * S                 # 512 tokens
FQ = HQ * HD              # 512 q features per core
NK = D // 128             # 32 contraction chunks
SCALE = 1.0 / float(np.sqrt(HD))

_CACHE = {}


def _build():
    import concourse.bass as bass
    import concourse.tile as tile
    from concourse import bacc, mybir

    F32 = mybir.dt.float32
    BF16 = mybir.dt.bfloat16
    AF = mybir.ActivationFunctionType

    nc = bacc.Bacc("TRN2", target_bir_lowering=False, debug=False,
                   enable_asserts=False, num_devices=NCORES)

    xs_d = nc.dram_tensor("xs", [128, NK * T], BF16, kind="ExternalInput").ap()
    wqkv_d = nc.dram_tensor("wqkv", [128, NK * 768], BF16, kind="ExternalInput").ap()
    wo_d = nc.dram_tensor("wo", [FQ, D], BF16, kind="ExternalInput").ap()
    cosT_d = nc.dram_tensor("cosT", [128, T], BF16, kind="ExternalInput").ap()
    sinT_d = nc.dram_tensor("sinT", [128, T], BF16, kind="ExternalInput").ap()
    mkT_d = nc.dram_tensor("mkT", [128, HQ * S], BF16, kind="ExternalInput").ap()
    ones_d = nc.dram_tensor("ones", [128, S], BF16, kind="ExternalInput").ap()
    identT_d = nc.dram_tensor("identT", [128, S], BF16, kind="ExternalInput").ap()
    pswap_d = nc.dram_tensor("pswap", [128, S], BF16, kind="ExternalInput").ap()
    bqT_d = nc.dram_tensor("bqT", [128, HQ], F32, kind="ExternalInput").ap()
    bkvT_d = nc.dram_tensor("bkvT", [128, 2], F32, kind="ExternalInput").ap()
    out_d = nc.dram_tensor("out", [T, D], BF16, kind="ExternalOutput").ap()

    # k-chunk DMA group sizes: small first groups so PE starts early
    GROUPS = [1, 1, 2, 4, 6, 6, 6, 6]
    assert sum(GROUPS) == NK

    with tile.TileContext(nc) as tc:
        with tc.tile_pool(name="consts", bufs=1) as cp:
            xs = cp.tile([128, NK * T], BF16)
            wq_s = cp.tile([128, NK * 768], BF16)
            wo_s = [cp.tile([128, D], BF16, name=f"wo{h}") for h in range(HQ)]
            cosT = cp.tile([128, T], BF16)
            sinT = cp.tile([128, T], BF16)
            mkT = cp.tile([128, HQ * S], BF16)
            ones = cp.tile([128, S], BF16)
            identT = cp.tile([128, S], BF16)
            pswap = cp.tile([128, S], BF16)
            bqT = cp.tile([128, HQ], F32)
            bkvT = cp.tile([128, 2], F32)

            # weights stream on the gpsimd (SWDGE) queue, x on the sync
            # (HWDGE-SP) queue; consts on scalar (HWDGE-ACT). Output stores
            # reuse SP later.
            k0 = 0
            for g, kg in enumerate(GROUPS):
                nc.gpsimd.dma_start(wq_s[:, k0 * 768:(k0 + kg) * 768],
                                    wqkv_d[:, k0 * 768:(k0 + kg) * 768])
                nc.sync.dma_start(xs[:, k0 * T:(k0 + kg) * T],
                                  xs_d[:, k0 * T:(k0 + kg) * T])
                k0 += kg
            for h in range(HQ):
                nc.gpsimd.dma_start(wo_s[h], wo_d[h * 128:(h + 1) * 128, :])
            nc.scalar.dma_start(cosT, cosT_d)
            nc.scalar.dma_start(sinT, sinT_d)
            nc.scalar.dma_start(mkT, mkT_d)
            nc.scalar.dma_start(ones, ones_d)
            nc.scalar.dma_start(identT, identT_d)
            nc.scalar.dma_start(pswap, pswap_d)
            nc.scalar.dma_start(bqT, bqT_d)
            nc.scalar.dma_start(bkvT, bkvT_d)

            with tc.tile_pool(name="sb", bufs=1) as sp, \
                 tc.tile_pool(name="tmp", bufs=2) as tp, \
                 tc.tile_pool(name="attn", bufs=2) as ap_, \
                 tc.tile_pool(name="aop", bufs=4) as aop, \
                 tc.tile_pool(name="outp", bufs=4) as op:

                # ---------- Phase A: QKV projections (transposed out) ------
                qTs = [sp.tile([128, T], BF16, name=f"qT{h}") for h in range(HQ)]
                kT_pre = sp.tile([128, T], BF16, name="kT_pre")
                vT = sp.tile([128, T], BF16, name="vT")
                with tc.tile_pool(name="psA", bufs=1, space="PSUM") as psA:
                    pq = [psA.tile([128, T], F32, tag=f"pq{h}", name=f"pq{h}")
                          for h in range(HQ)]
                    pk = psA.tile([128, T], F32, tag="pk", name="pk")
                    pv = psA.tile([128, T], F32, tag="pv", name="pv")
                    for k in range(NK):
                        rhs = xs[:, k * T:(k + 1) * T]
                        base = k * 768
                        st = (k == 0)
                        sp_ = (k == NK - 1)
                        for h in range(HQ):
                            nc.tensor.matmul(
                                pq[h], wq_s[:, base + h * 128:base + (h + 1) * 128],
                                rhs, start=st, stop=sp_)
                        nc.tensor.matmul(pk, wq_s[:, base + 512:base + 640],
                                         rhs, start=st, stop=sp_)
                        nc.tensor.matmul(pv, wq_s[:, base + 640:base + 768],
                                         rhs, start=st, stop=sp_)
                    # PSUM -> SBUF with bias add + bf16 cast (ACT/DVE split)
                    for h in range(HQ):
                        if h % 2 == 0:
                            nc.scalar.activation(qTs[h], pq[h],
                                                 AF.Identity, bias=bqT[:, h:h + 1])
                        else:
                            nc.vector.tensor_scalar_add(qTs[h], pq[h],
                                                        bqT[:, h:h + 1])
                    nc.vector.tensor_scalar_add(vT, pv, bkvT[:, 1:2])
                    nc.scalar.activation(kT_pre, pk, AF.Identity, bias=bkvT[:, 0:1])

                    # ------ Phase B: RoPE (transposed) + v transposes ------
                    # stays inside the psA pool: the 2 swp-tag banks plus the
                    # six accumulation banks fill PSUM exactly, and swap MMs
                    # can start as soon as their head's bias copy lands (no
                    # pool-close barrier).
                    qrT = sp.tile([128, HQ * T], BF16, name="qrT")
                    krT = sp.tile([128, T], BF16, name="krT")
                    v_m = [None] * B
                    for h in range(HQ + 1):
                        src = (qTs[h] if h < HQ else kT_pre)
                        dst = (qrT[:, h * T:(h + 1) * T] if h < HQ else krT)
                        pswp = psA.tile([128, T], F32, tag="swp", bufs=2,
                                        name=f"pswp{h}")
                        nc.tensor.matmul(pswp, pswap, src, start=True, stop=True)
                        # spread the RoPE elementwise work: ACT drains PSUM,
                        # Pool does the cos mul, DVE (4x bf16) the sin mul+add
                        t2s = tp.tile([128, T], BF16, tag="t2s", name=f"t2s{h}")
                        nc.scalar.copy(t2s, pswp)
                        t1 = tp.tile([128, T], BF16, tag="t1", name=f"t1_{h}")
                        nc.gpsimd.tensor_mul(t1, src, cosT)
                        t2 = tp.tile([128, T], BF16, tag="t2", name=f"t2_{h}")
                        nc.vector.tensor_mul(t2, t2s, sinT)
                        nc.vector.tensor_add(dst, t1, t2)
                    pvm = psA.tile([128, T], BF16, tag="swp", bufs=2,
                                   name="pvm")
                    for m in range(B):
                        nc.tensor.transpose(pvm[:, m * S:(m + 1) * S],
                                            vT[:, m * S:(m + 1) * S], identT)
                    v_all = sp.tile([128, T], BF16, name="v_all")
                    nc.vector.tensor_copy(v_all, pvm)
                    for m in range(B):
                        v_m[m] = v_all[:, m * S:(m + 1) * S]

                with tc.tile_pool(name="psB", bufs=1, space="PSUM") as psB:
                    # ---------- Phase C: attention per batch ----------
                    # po_u = V^T @ (exp(scores) * mask) runs in parallel with
                    # the denominator matmul; normalization folds into the
                    # PSUM->SBUF copy (rec rows are all equal).
                    qv = qrT.rearrange("p (h t) -> p h t", h=HQ)
                    aoT = [None] * B
                    for m in range(B):
                        psc = psB.tile([128, HQ * S], F32, tag="psc", bufs=2,
                                       name=f"psc{m}")
                        nc.tensor.matmul(psc, krT[:, m * S:(m + 1) * S],
                                         qv[:, :, m * S:(m + 1) * S],
                                         start=True, stop=True)
                        eu = ap_.tile([128, HQ * S], BF16, tag="eu", name=f"eu{m}")
                        nc.scalar.activation(eu, psc, AF.Exp, scale=SCALE)
                        au = ap_.tile([128, HQ * S], BF16, tag="au", name=f"au{m}")
                        nc.gpsimd.tensor_mul(au, eu, mkT)
                        pden = psB.tile([128, HQ * S], F32, tag="pden",
                                        name=f"pden{m}")
                        nc.tensor.matmul(pden, ones, au, start=True, stop=True)
                        po = psB.tile([128, HQ * S], F32, tag="po", name=f"po{m}")
                        nc.tensor.matmul(po, v_m[m], au, start=True, stop=True)
                        rec = ap_.tile([128, HQ * S], F32, tag="rec", name=f"rec{m}")
                        nc.vector.reciprocal(rec, pden)
                        aoT[m] = aop.tile([128, HQ * S], BF16, tag="aoT",
                                          name=f"aoT{m}")
                        nc.vector.tensor_mul(aoT[m], po, rec)

                    # ---------- Phase D: output projection ----------
                    NT = D // 512
                    for m in range(B):
                        outm = op.tile([128, D], BF16, tag="outm", name=f"outm{m}")
                        for n in range(NT):
                            pso = psB.tile([128, 512], F32, tag="pso", bufs=2,
                                           name=f"pso{m}_{n}")
                            for h in range(HQ):
                                nc.tensor.matmul(
                                    pso, aoT[m][:, h * 128:(h + 1) * 128],
                                    wo_s[h][:, n * 512:(n + 1) * 512],
                                    start=(h == 0), stop=(h == HQ - 1))
                            if (m * NT + n) % 2 == 0:
                                nc.vector.tensor_copy(
                                    outm[:, n * 512:(n + 1) * 512], pso)
                            else:
                                nc.scalar.copy(
                                    outm[:, n * 512:(n + 1) * 512], pso)
                            if m == B - 1:
                                # last batch: store per n-tile to shorten the
                                # final copy->DMA->drain chain
                                nc.sync.dma_start(
                                    out_d[m * S:(m + 1) * S,
                                          n * 512:(n + 1) * 512],
                                    outm[:, n * 512:(n + 1) * 512])
                            elif n % 2 == 1:
                                # store finished 1024-col quarter immediately
                                qlo = (n - 1) * 512
                                nc.sync.dma_start(
                                    out_d[m * S:(m + 1) * S, qlo:qlo + 1024],
                                    outm[:, qlo:qlo + 1024])

    nc.compile()
    return nc


def _prep_inputs(x, freqs_cos, freqs_sin, wq, bq, wk, bk, wv, bv, wo):
    from ml_dtypes import bfloat16 as bf16

    xT = x.reshape(T, D).T.astype(np.float32)                  # (D, T)
    xs = np.ascontiguousarray(
        xT.reshape(NK, 128, T).transpose(1, 0, 2).reshape(128, NK * T)
    ).astype(bf16)

    cos_d = np.repeat(freqs_cos.astype(np.float32), 2, axis=1)  # (S, 128)
    sin_d = np.repeat(freqs_sin.astype(np.float32), 2, axis=1)
    sign = np.tile(np.array([-1.0, 1.0], np.float32), HD // 2)
    cosT = np.ascontiguousarray(np.tile(cos_d.T, (1, B))).astype(bf16)  # (128, T)
    sinT = np.ascontiguousarray(
        np.tile((sin_d * sign[None, :]).T, (1, B))).astype(bf16)
    mkT = np.ascontiguousarray(
        np.tile(np.triu(np.ones((S, S), np.float32)), (1, HQ))).astype(bf16)
    ones = np.ones((S, S), np.float32).astype(bf16)
    identT = np.eye(S, dtype=np.float32).astype(bf16)
    pswap = np.kron(np.eye(HD // 2, dtype=np.float32),
                    np.array([[0, 1], [1, 0]], np.float32)).astype(bf16)

    maps = []
    for c in range(NCORES):
        qs = slice(c * FQ, (c + 1) * FQ)
        ks = slice(c * HD, (c + 1) * HD)
        wqkv = np.concatenate(
            [wq[:, qs], wk[:, ks], wv[:, ks]], axis=1).astype(np.float32)  # (D, 768)
        wqkv_t = np.ascontiguousarray(
            wqkv.reshape(NK, 128, 768).transpose(1, 0, 2).reshape(128, NK * 768)
        ).astype(bf16)
        bqT = np.ascontiguousarray(
            bq[qs].astype(np.float32).reshape(HQ, HD).T)       # (128, HQ)
        bkvT = np.ascontiguousarray(
            np.stack([bk[ks], bv[ks]], axis=1).astype(np.float32))  # (128, 2)
        maps.append({
            "xs": xs,
            "wqkv": wqkv_t,
            "wo": np.ascontiguousarray(wo[qs, :].astype(np.float32)).astype(bf16),
            "cosT": cosT, "sinT": sinT, "mkT": mkT, "ones": ones,
            "identT": identT, "pswap": pswap, "bqT": bqT, "bkvT": bkvT,
        })
    return maps


def kernel(x, start_pos, freqs_cos, freqs_sin, mask, cache_k, cache_v,
           wq, bq, wk, bk, wv, bv, wo, bo, _want_trace=False):
    from concourse.bass_utils import run_bass_kernel_spmd

    assert int(start_pos) == 0
    if "nc" not in _CACHE:
        _CACHE["nc"] = _build()
    nc = _CACHE["nc"]
    in_maps = _prep_inputs(np.asarray(x), np.asarray(freqs_cos),
                           np.asarray(freqs_sin), np.asarray(wq),
                           np.asarray(bq), np.asarray(wk), np.asarray(bk),
                           np.asarray(wv), np.asarray(bv), np.asarray(wo))
    res = run_bass_kernel_spmd(nc, in_maps, core_ids=list(range(NCORES)),
                               trace=_want_trace)
    acc = np.zeros((T, D), np.float64)
    for r in res.results:
        acc += r["out"].astype(np.float64)
    out = (acc + np.asarray(bo).astype(np.float64)).astype(np.float32)
    if _want_trace:
        _CACHE["last_exec_time_ns"] = res.exec_time_ns
        _CACHE["last_trace"] = res.instructions_and_trace
    return out.reshape(B, S, D)
